# revision 8
# baseline (speedup 1.0000x reference)
"""CAN per-sample 2-layer MLP kernel for Trainium2 (8 NeuronCores, SPMD).

Computation (per sample b):
    x = user_emb[b]                           # (50, 16)
    W0, b0, W1, b1 unpacked from item_emb[b]  # (16,16),(16,),(16,16),(16,)
    y = relu(relu(x @ W0 + b0) @ W1 + b1)     # (50, 16)

Mapping:
  * Pure data parallel over 8 cores; each core gets 2048 samples padded
    to 2112 (zero samples, discarded on unpack) so the grouping divides.
  * fp16 throughout the PE path (fp32 runs 2 matmul passes; fp16 is one
    pass and rel-err ~6e-4, well inside tolerance). PSUM accumulates
    fp32; output is stored fp16.
  * Host packs x^T per sample with an appended ones-row (homogeneous
    coordinates); bias folds into a 17x17 Wt0 = [[W0,0],[b0,1]] and a
    17x16 Wt1 = [[W1],[b1]] so `x_t @ Wt` applies bias, and the ones row
    self-propagates through layer 1 (relu(1)=1).
  * Samples are grouped in THREES: one group is a K=51 matmul with a
    block-diagonal 51x51 Wt0 (and 51x48 Wt1). Two groups per
    128-partition tile at 64-row strides run at PE tile_positions
    (64j2,64j2): LDWEIGHTS of one quadrant overlaps MATMUL of the other,
    so the PE stream is bound by max(LDW, MM) per instruction. m=3 makes
    LDW(51 rows) ~ MM(50 moving rows).
  * The block-diagonal ZEROS are never DMAed. Weight blocks land
    INTERLEAVED at group stride G: stationary column j of group q lives
    at SBUF col G*j+q, so the matmul stationary is a 2D strided AP
    [[row,51],[G,51]] while each lane's blocks still form one contiguous
    2D DMA run ([17 rows, 1, G*blk]) on partition rows 17t..17t+16. The
    off-lane rows of each strip region are zeroed ONCE per pool buffer
    at startup and never rewritten (W DMAs never touch them), which
    supplies the block-diagonal zeros for free. Cuts input DMA by 44%.
  * Input DMAs are issued 3 batches ahead via SWDGE on GpSimd (HWDGE
    pins DRAM-sourced loads to one SDMA engine); outputs stream via the
    Sync queue. Relus are split across ACT and DVE (173/227 balances
    their fixed overheads). The PE stream is software-pipelined with a
    one-sub-batch skew so relu1 latency hides under the next sub-batch's
    layer-1 matmuls.
  * Walrus codegen caps inline sync waits (DMACopy/Matmult: 1): a post-pass
    moves excess waits onto NoOps inserted before the instruction on the
    same queue. SBUF-side DMA APs must stay 2D ([row, nparts], [1, run]) -
    the DIRECT2D DMA struct cannot roll a free run across partitions.
"""

from contextlib import ExitStack

import numpy as np

import concourse.bass as bass
import concourse.mybir as mybir
from concourse import tile
from concourse.bass_utils import run_bass_kernel_spmd
from concourse.tile_rust import add_dep_helper

# Problem constants (hardcoded per contract)
B, N, D = 16384, 50, 16
NCORES = 8
K = D + 1                    # 17 rows per sample: 16 features + ones row
M3 = 3                       # samples per matmul group (block-diagonal)
KG = M3 * K                  # 51 contraction rows per group
MG2 = M3 * D                 # 48 layer-2 output rows per group
SPC = 2112                   # samples per core, padded (2048 real)
NGP = SPC // (2 * M3)        # 352 group-pairs per core
G = 32                       # group-pairs per DMA batch
NB = NGP // G                # 11 batches
GS = 8                       # group-pairs per PSUM sub-batch (one bank)
SW = K + D                   # 33 strip cols per group: Wt0 blk | Wt1 blk

F32 = mybir.dt.float32
F16 = mybir.dt.float16

AC = 173                     # relu cols for ACT; DVE gets sf-AC (balances fixed costs)
PF = 3                       # input prefetch depth (batches)
NWBUF = 4                    # wpool depth (= number of startup zero-fills)


def _strip_covered_waits(nc):
    """Remove, from DMACopy instructions, semaphore waits already guaranteed
    by an earlier instruction on the same engine queue. Coverage is killed
    for a sem from the point of any non-increment update (barrier resets)."""
    for fn in nc.m.functions:
        for blk in fn.blocks:
            seen = {}
            for ins in blk.instructions:
                si = ins.sync_info
                if si is None:
                    continue
                eng = ins.engine
                strippable = type(ins).__name__ == "InstDMACopy"
                kept = []
                changed = False
                for w in si.on_wait:
                    if (
                        strippable
                        and w.wait_mode == "sem-ge-imm"
                        and w.wait_reg is None
                        and seen.get((eng, w.id), -1) >= w.wait_value
                    ):
                        changed = True
                        continue
                    kept.append(w)
                for w in kept:
                    if w.wait_mode == "sem-ge-imm" and w.wait_reg is None:
                        key = (eng, w.id)
                        if seen.get(key, -1) < w.wait_value:
                            seen[key] = w.wait_value
                for u in si.on_update:
                    if u.update_mode != "sem-add-imm" or (
                        u.update_value is not None and u.update_value < 0
                    ):
                        for key in [k for k in seen if k[1] == u.id]:
                            del seen[key]
                if changed:
                    ins.sync_info = mybir.SyncInfo(
                        on_wait=kept, on_update=si.on_update
                    )


_WS_COUNT = [0]


def _split_excess_waits(nc, cap=1):
    """Move excess inline waits onto NoOps inserted immediately before, on
    the same engine queue - semantically identical (sequencers execute
    waits in order)."""
    for fn in nc.m.functions:
        for blk in fn.blocks:
            insts = blk.instructions
            i = 0
            while i < len(insts):
                ins = insts[i]
                si = ins.sync_info
                if si is None or len(si.on_wait) <= cap:
                    i += 1
                    continue
                waits = list(si.on_wait)
                keep, extra = waits[-cap:], waits[:-cap]
                ins.sync_info = mybir.SyncInfo(on_wait=keep, on_update=si.on_update)
                for w in extra:
                    _WS_COUNT[0] += 1
                    nop = mybir.InstNoOp(name=f"I-ws{_WS_COUNT[0]}", ins=[], outs=[])
                    nop.engine = ins.engine
                    nop.sync_info = mybir.SyncInfo(on_wait=[w], on_update=[])
                    insts.insert(i, nop)
                    i += 1
                i += 1


def build_nc(g=G, gs=GS, dt=F16, sim_mode=False):
    """Build the per-core Bass program.

    DRAM (per core), batch-major:
      xh  [NB, 2*KG, g*N] : row KG*j2+r, col qq*N+n   = x^T of group (bi*g+qq)*2+j2
      w0h [NB, 6*K, g*K]  : row (j2*3+t)*17+r, col i*g+q = Wt0[r,i] of lane t, group q
      w1h [NB, 6*K, g*D]  : same, Wt1 blocks
      yh  [NB, 2*MG2, g*N]: row MG2*j2+(16l+e), col qq*N+n = group outputs
    """
    nbatch = NB
    assert g % gs == 0
    nsub = g // gs
    xf = N * g               # x / yt data cols per batch
    sf = N * gs              # psum cols per sub-batch
    xfp = xf + 8             # padded row widths: keep SBUF DMA APs 2D
    wfp = SW * M3 * g + 8    # 99g: L1 region (51g) then L2 region (48g)
    L2O = KG * g             # col offset of the L2 region

    nc = bass.Bass(
        "TRN2",
        target_bir_lowering=False,
        debug=False,
        detect_race_conditions=False,  # post-pass NoOps confuse its bookkeeping
    )
    xh = nc.dram_tensor("xh", [nbatch, 2 * KG, xf], dt, kind="ExternalInput")
    w0h = nc.dram_tensor("w0h", [nbatch, 6 * K, K * g], dt, kind="ExternalInput")
    w1h = nc.dram_tensor("w1h", [nbatch, 6 * K, D * g], dt, kind="ExternalInput")
    yh = nc.dram_tensor("yh", [nbatch, 2 * MG2, xf], dt, kind="ExternalOutput")

    relu = mybir.ActivationFunctionType.Relu

    with tile.TileContext(nc) as tc, ExitStack() as ctx:
        xpool = ctx.enter_context(tc.tile_pool(name="xpool", bufs=PF + 2))
        wpool = ctx.enter_context(tc.tile_pool(name="wpool", bufs=NWBUF))
        hpool = ctx.enter_context(tc.tile_pool(name="hpool", bufs=3))
        ypool = ctx.enter_context(tc.tile_pool(name="ypool", bufs=3))
        pspool = ctx.enter_context(tc.tile_pool(name="ps", bufs=4, space="PSUM"))

        prev_sp = [None]

        def sp_chain(inst):
            # Pin SP issue order to emission order so prefetched loads are
            # dispatched before later batches' stores.
            if prev_sp[0] is not None:
                add_dep_helper(inst.ins, prev_sp[0].ins, sync=False,
                               reason="SP issue order")
            prev_sp[0] = inst
            return inst

        cts = {}

        def emit_in_dma(bi):
            xt = xpool.tile([128, xfp], dt, name="xt")
            wt = wpool.tile([128, wfp], dt, name="wt")
            cts[bi] = (xt, wt)
            # SWDGE: HWDGE pins DRAM-sourced loads to one SDMA engine;
            # SWDGE sprays descriptors across engines by dest partition.
            for j2 in range(2):
                nc.gpsimd.dma_start(
                    bass.AP(xt.tensor, 64 * j2 * xfp, [[xfp, KG], [1, xf]]),
                    bass.AP(xh, (bi * 2 + j2) * KG * xf, [[xf, KG], [1, xf]]),
                )
            # Zero-fill each strip buffer ONCE (first rotation): W DMAs only
            # ever write the 17 lane rows of each strip; off-lane rows stay
            # zero forever, giving the block-diagonal zeros without DMAing
            # them. Emitted after the x loads so those issue immediately;
            # the W DMAs below auto-depend on the fill.
            if bi < NWBUF:
                h = wfp // 2
                nc.vector.memset(wt[:, :h], 0.0)
                nc.gpsimd.memset(wt[:, h:], 0.0)
            for j2 in range(2):
                for t in range(M3):
                    nc.gpsimd.dma_start(
                        bass.AP(wt.tensor, (64 * j2 + K * t) * wfp + K * g * t,
                                [[wfp, K], [1, K * g]]),
                        bass.AP(w0h, ((bi * 2 + j2) * M3 + t) * K * (K * g),
                                [[K * g, K], [1, K * g]]),
                    )
                    nc.gpsimd.dma_start(
                        bass.AP(wt.tensor, (64 * j2 + K * t) * wfp + L2O + D * g * t,
                                [[wfp, K], [1, D * g]]),
                        bass.AP(w1h, ((bi * 2 + j2) * M3 + t) * K * (D * g),
                                [[D * g, K], [1, D * g]]),
                    )

        # deep prefetch prologue
        for pb in range(min(PF, nbatch)):
            emit_in_dma(pb)

        # Software-pipelined emission with one-sub-batch skew: the PE queue
        # is in-order, so L2(s) (which waits on relu1(s)) is emitted AFTER
        # L1(s+1) - the PE computes layer 1 of the next sub-batch while
        # relu1(s) runs on ACT, instead of stalling.
        subs = [(bi, s) for bi in range(nbatch) for s in range(nsub)]
        state = {}   # ss -> (xt, wt, ht)
        yts = {}

        def emit_l1(ss):
            bi, s = subs[ss]
            if s == 0:
                if bi + PF < nbatch:
                    emit_in_dma(bi + PF)
                yts[bi] = ypool.tile([128, xfp], dt, name="yt")
            xt, wt = cts[bi]
            ps1 = pspool.tile([128, sf], F32, name="ps1")
            if sim_mode:
                nc.vector.memset(ps1[:, :], 0.0)
            for q in range(gs):
                qq = s * gs + q
                for j2 in range(2):
                    nc.tensor.matmul(
                        bass.AP(ps1.tensor, 64 * j2 * sf + q * N, [[sf, KG], [1, N]]),
                        bass.AP(wt.tensor, 64 * j2 * wfp + qq,
                                [[wfp, KG], [g, KG]]),
                        bass.AP(xt.tensor, 64 * j2 * xfp + qq * N, [[xfp, KG], [1, N]]),
                        start=True,
                        stop=True,
                        tile_position=(64 * j2, 64 * j2),
                    )
            ht = hpool.tile([128, sf], dt, name="ht")
            # split relu1 across ACT+DVE: halves the latency on the
            # layer-2 critical path; subtile deps let each half's L2 matmuls
            # start independently
            nc.scalar.activation(ht[:, :AC], ps1[:, :AC], relu)
            nc.vector.tensor_scalar_max(ht[:, AC:], ps1[:, AC:], 0.0)
            state[ss] = (xt, wt, ht)

        def emit_l2(ss):
            bi, s = subs[ss]
            xt, wt, ht = state.pop(ss)
            yt = yts[bi]
            ps2 = pspool.tile([128, sf], F32, name="ps2")
            if sim_mode:
                nc.vector.memset(ps2[:, :], 0.0)
            for q in range(gs):
                qq = s * gs + q
                for j2 in range(2):
                    nc.tensor.matmul(
                        bass.AP(ps2.tensor, 64 * j2 * sf + q * N, [[sf, MG2], [1, N]]),
                        bass.AP(wt.tensor, 64 * j2 * wfp + L2O + qq,
                                [[wfp, KG], [g, MG2]]),
                        bass.AP(ht.tensor, 64 * j2 * sf + q * N, [[sf, KG], [1, N]]),
                        start=True,
                        stop=True,
                        tile_position=(64 * j2, 64 * j2),
                    )
            # relu2 split across DVE+ACT
            nc.vector.tensor_scalar_max(
                bass.AP(yt.tensor, s * sf, [[xfp, 128], [1, sf - AC]]),
                ps2[:, :sf - AC],
                0.0,
            )
            nc.scalar.activation(
                bass.AP(yt.tensor, s * sf + sf - AC, [[xfp, 128], [1, AC]]),
                ps2[:, sf - AC:],
                relu,
            )
            if s == nsub - 1:
                for j2 in range(2):
                    sp_chain(nc.sync.dma_start(
                        bass.AP(yh, (bi * 2 + j2) * MG2 * xf, [[xf, MG2], [1, xf]]),
                        bass.AP(yt.tensor, 64 * j2 * xfp, [[xfp, MG2], [1, xf]]),
                    ))
                cts.pop(bi)

        # skew 1: L2(s) is emitted after L1(s+1) so the next sub-batch of
        # layer-1 matmuls covers the relu1 chain latency on the in-order PE queue
        SKEW = 1
        for idx in range(len(subs) + SKEW):
            if idx < len(subs):
                emit_l1(idx)
            if idx >= SKEW:
                emit_l2(idx - SKEW)

    _strip_covered_waits(nc)
    _split_excess_waits(nc)
    return nc


def pack_inputs(user_emb, item_emb, dt=np.float16):
    """Shard + lay out inputs for the 8 cores (3-sample group layout)."""
    x = np.ascontiguousarray(user_emb, dtype=np.float32)
    ie = np.ascontiguousarray(item_emb, dtype=np.float32)
    bpc = B // NCORES                       # 2048 real samples per core
    S = NCORES * SPC                        # padded total

    # per-core zero padding to SPC samples
    xs = np.zeros((S, N, D), dtype=np.float32)
    ws = np.zeros((S, ie.shape[1]), dtype=np.float32)
    xs.reshape(NCORES, SPC, N, D)[:, :bpc] = x.reshape(NCORES, bpc, N, D)
    ws.reshape(NCORES, SPC, -1)[:, :bpc] = ie.reshape(NCORES, bpc, -1)

    # x^T with ones row: [S, 17, 50]
    xt = np.empty((S, K, N), dtype=np.float32)
    xt[:, :D] = xs.transpose(0, 2, 1)
    xt[:, D] = 1.0

    # Wt0 [S,17,17] (bias row + ones col), Wt1 [S,17,16] (bias row)
    w0 = np.zeros((S, K, K), dtype=np.float32)
    w0[:, :D, :D] = ws[:, : D * D].reshape(S, D, D)
    w0[:, D, :D] = ws[:, D * D : D * D + D]             # b0
    w0[:, D, D] = 1.0                                   # ones col
    off = D * (D + 1)
    w1 = np.empty((S, K, D), dtype=np.float32)
    w1[:, :D, :] = ws[:, off : off + D * D].reshape(S, D, D)
    w1[:, D, :] = ws[:, off + D * D : off + D * D + D]  # b1

    # xh: groups of 3 stack their x^T rows: [c, bi, j2, 51, G, 50]
    xh = (
        xt.reshape(NCORES, NB, G, 2, KG, N)
        .transpose(0, 1, 3, 4, 2, 5)
        .astype(dt, copy=False)
        .reshape(NCORES, NB, 2 * KG, G * N)
    )
    # weight strips, group-interleaved: col i*G+q so the stationary AP is
    # a single strided free dim [[row,51],[G,51]]
    w0h = (
        w0.reshape(NCORES, NB, G, 2, M3, K, K)
        .transpose(0, 1, 3, 4, 5, 6, 2)      # c, bi, j2, t, row, i, q
        .astype(dt, copy=False)
        .reshape(NCORES, NB, 6 * K, K * G)
    )
    w1h = (
        w1.reshape(NCORES, NB, G, 2, M3, K, D)
        .transpose(0, 1, 3, 4, 5, 6, 2)
        .astype(dt, copy=False)
        .reshape(NCORES, NB, 6 * K, D * G)
    )
    return [
        {
            "xh": np.ascontiguousarray(xh[c]),
            "w0h": np.ascontiguousarray(w0h[c]),
            "w1h": np.ascontiguousarray(w1h[c]),
        }
        for c in range(NCORES)
    ]


def unpack_output(results):
    """results: per-core {"yh": [NB, 96, G*N]} -> full (B, N, D) f32."""
    bpc = B // NCORES
    yh = np.stack([np.asarray(r["yh"], dtype=np.float32) for r in results])
    y = (
        yh.reshape(NCORES, NB, 2, M3, D, G, N)   # c, bi, j2, l, e, qq, n
        .transpose(0, 1, 5, 2, 3, 6, 4)          # c, bi, qq, j2, l, n, e
        .reshape(NCORES, SPC, N, D)
    )
    return np.ascontiguousarray(y[:, :bpc].reshape(B, N, D))


_NC_CACHE = {}


def _get_nc(key=(G, GS)):
    if key not in _NC_CACHE:
        g, gs = key
        _NC_CACHE[key] = build_nc(g=g, gs=gs)
    return _NC_CACHE[key]


def kernel(user_emb, item_emb):
    nc = _get_nc()
    in_maps = pack_inputs(user_emb, item_emb)
    res = run_bass_kernel_spmd(nc, in_maps, core_ids=list(range(NCORES)))
    return unpack_output(res.results)
